# revision 2
# baseline (speedup 1.0000x reference)
"""Trainium2 Bass kernel for nn_Memory (topk_masking).

Algorithm (per query row q of N=32768, item count 2048, K=10):
  logits l = q @ mempool.T
  e = exp(l); S = sum(e); att p = e/S              (softmax, no max-shift: |l| <= ~3)
  top-10 selection on e via DVE max8 + match_replace + max8 -> t10 (10th largest e)
  u = exp(e/S)  (= exp(att));  g = (e >= t10) * u;  Z = sum(g)
  out = (g @ mempool) / Z      (masked dense matmul instead of gather)

Precision: logits via 3-term fp16 split (qh@mh + qh@ml + ql@mh) -> ~fp32-grade
(sigma ~1e-7), so the selected top-10 set matches the fp32 reference.
Second matmul in fp16 (g and mempool) -> ~5e-4 relative output error.

Sharding: data-parallel over the flattened query dim. 32 units of
[512 dim x 1024 queries] (16 batches x 2 inputs); each of 8 cores takes 4 units
= 4096 queries = 32 tiles of 128 queries. mempool (4MB) replicated per core.

Host does layout marshalling only: fp16 hi/lo splits, transposes, reassembly.
"""
import sys
sys.path.insert(0, '/opt/trn_rl_repo')

import numpy as np
import concourse.bacc as bacc
import concourse.mybir as mybir
import concourse.tile as tile
from concourse.bass_utils import run_bass_kernel_spmd

F32 = mybir.dt.float32
F16 = mybir.dt.float16
BF16 = mybir.dt.bfloat16

DIM = 512
NITEM = 2048
NCORES = 8
UNITS_PER_CORE = 4
QPU = 1024                      # queries per unit
TILES = UNITS_PER_CORE * QPU // 128   # 32 tiles of 128 queries per core
NEG = -1e30
EXP = mybir.ActivationFunctionType.Exp
COPY = mybir.ActivationFunctionType.Copy

_prog_cache = {}


def build_program():
    if 'nc' in _prog_cache:
        return _prog_cache['nc']
    nc = bacc.Bacc()

    qh_d = nc.declare_dram_parameter("qh", [UNITS_PER_CORE, DIM, QPU], F16, isOutput=False)
    ql_d = nc.declare_dram_parameter("ql", [UNITS_PER_CORE, DIM, QPU], F16, isOutput=False)
    mh_d = nc.declare_dram_parameter("mh", [DIM, NITEM], F16, isOutput=False)   # mempool.T hi
    ml_d = nc.declare_dram_parameter("ml", [DIM, NITEM], F16, isOutput=False)   # mempool.T lo
    mp_d = nc.declare_dram_parameter("mp", [NITEM, DIM], F16, isOutput=False)   # mempool
    id_d = nc.declare_dram_parameter("ident", [128, 128], F16, isOutput=False)
    out_d = nc.declare_dram_parameter("out", [UNITS_PER_CORE * QPU, DIM], F32, isOutput=True)

    with tile.TileContext(nc) as tc:
        with (
            tc.tile_pool(name="const", bufs=1) as cpool,
            tc.tile_pool(name="qin", bufs=3) as qpool,
            tc.tile_pool(name="work", bufs=2) as wpool,
            tc.tile_pool(name="outp", bufs=3) as opool,
            tc.tile_pool(name="ps_l", bufs=2, space="PSUM") as ps_l,
            tc.tile_pool(name="ps_t", bufs=2, space="PSUM") as ps_t,
            tc.tile_pool(name="ps_o", bufs=2, space="PSUM") as ps_o,
        ):
            # persistent weights
            mh_sb = cpool.tile([128, 4, NITEM], F16)
            ml_sb = cpool.tile([128, 4, NITEM], F16)
            mp_sb = cpool.tile([128, 16, DIM], F16)
            id_sb = cpool.tile([128, 128], F16)
            nc.sync.dma_start(mh_sb[:], mh_d[:].rearrange("(kc p) n -> p kc n", p=128))
            nc.sync.dma_start(ml_sb[:], ml_d[:].rearrange("(kc p) n -> p kc n", p=128))
            nc.sync.dma_start(mp_sb[:], mp_d[:].rearrange("(ic p) d -> p ic d", p=128))
            nc.sync.dma_start(id_sb[:], id_d[:])

            for t in range(TILES):
                u, tt = divmod(t, QPU // 128)
                qh_sb = qpool.tile([128, 4, 128], F16, tag="qh")
                ql_sb = qpool.tile([128, 4, 128], F16, tag="ql")
                nc.sync.dma_start(
                    qh_sb[:],
                    qh_d[u, :, 128 * tt:128 * (tt + 1)].rearrange("(kc p) f -> p kc f", p=128))
                nc.sync.dma_start(
                    ql_sb[:],
                    ql_d[u, :, 128 * tt:128 * (tt + 1)].rearrange("(kc p) f -> p kc f", p=128))

                e_sb = wpool.tile([128, NITEM], F32, tag="e")
                S_p = wpool.tile([128, 2], F32, tag="Sp")
                # ---- mm1: l = q @ mempool.T via 3-term fp16 split, 2 halves ----
                for h in range(2):
                    l_ps = ps_l.tile([128, 1024], F32, tag="l")
                    for kc in range(4):
                        for c2 in range(2):
                            col = 1024 * h + 512 * c2
                            dst = l_ps[:, 512 * c2:512 * (c2 + 1)]
                            nc.tensor.matmul(dst, qh_sb[:, kc, :],
                                             mh_sb[:, kc, col:col + 512],
                                             start=(kc == 0), stop=False)
                            nc.tensor.matmul(dst, qh_sb[:, kc, :],
                                             ml_sb[:, kc, col:col + 512],
                                             start=False, stop=False)
                    for kc in range(4):
                        for c2 in range(2):
                            col = 1024 * h + 512 * c2
                            dst = l_ps[:, 512 * c2:512 * (c2 + 1)]
                            nc.tensor.matmul(dst, ql_sb[:, kc, :],
                                             mh_sb[:, kc, col:col + 512],
                                             start=False, stop=(kc == 3))
                    # e = exp(l), with per-half accumulated row-sum
                    nc.scalar.activation(e_sb[:, 1024 * h:1024 * (h + 1)], l_ps[:],
                                         EXP, accum_out=S_p[:, h:h + 1])

                S_sb = wpool.tile([128, 1], F32, tag="S")
                nc.vector.tensor_add(S_sb[:], S_p[:, 0:1], S_p[:, 1:2])

                # ---- top-10 threshold: ranks 1-8 then 9-16 ----
                top8 = wpool.tile([128, 8], F32, tag="top8")
                next8 = wpool.tile([128, 8], F32, tag="next8")
                em_sb = wpool.tile([128, NITEM], F32, tag="em")
                nc.vector.max(out=top8[:], in_=e_sb[:])
                nc.vector.match_replace(out=em_sb[:], in_to_replace=top8[:],
                                        in_values=e_sb[:], imm_value=NEG)
                nc.vector.max(out=next8[:], in_=em_sb[:])

                # ---- u = exp(e/S); g = (e >= t10) * u; Z = sum(g) ----
                Sinv = wpool.tile([128, 1], F32, tag="Sinv")
                nc.vector.reciprocal(Sinv[:], S_sb[:])
                u_sb = wpool.tile([128, NITEM], F16, tag="u")
                nc.scalar.activation(u_sb[:], e_sb[:], EXP, scale=Sinv[:])

                g_sb = wpool.tile([128, NITEM], F16, tag="g")
                Z_sb = wpool.tile([128, 1], F32, tag="Z")
                nc.vector.scalar_tensor_tensor(
                    out=g_sb[:], in0=e_sb[:], scalar=next8[:, 1:2], in1=u_sb[:],
                    op0=mybir.AluOpType.is_ge, op1=mybir.AluOpType.mult,
                    accum_out=Z_sb[:])
                Zinv = wpool.tile([128, 1], F32, tag="Zinv")
                nc.vector.reciprocal(Zinv[:], Z_sb[:])

                # ---- transpose g -> gT (16 PE transposes via 4 psum quarters) ----
                gt_sb = wpool.tile([128, 16, 128], F16, tag="gt")
                for qd in range(4):
                    gt_ps = ps_t.tile([128, 512], F16, tag="gt")
                    for b in range(4):
                        blk = 4 * qd + b
                        nc.tensor.transpose(gt_ps[:, 128 * b:128 * (b + 1)],
                                            g_sb[:, 128 * blk:128 * (blk + 1)], id_sb[:])
                    nc.scalar.copy(out=gt_sb[:, 4 * qd:4 * (qd + 1), :], in_=gt_ps[:])

                # ---- mm2: out = gT.T @ mempool, then scale rows by 1/Z ----
                o_ps = ps_o.tile([128, DIM], F32, tag="o")
                for ic in range(16):
                    nc.tensor.matmul(o_ps[:], gt_sb[:, ic, :], mp_sb[:, ic, :],
                                     start=(ic == 0), stop=(ic == 15))
                o_sb = opool.tile([128, DIM], F32, tag="osb")
                nc.scalar.activation(o_sb[:], o_ps[:], COPY, scale=Zinv[:])
                nc.sync.dma_start(out_d[128 * t:128 * (t + 1), :], o_sb[:])

    nc.finalize()
    _prog_cache['nc'] = nc
    return nc


def _prep_inputs(input1, input2, mempool):
    units = np.concatenate([
        np.asarray(input1, dtype=np.float32).reshape(16, DIM, QPU),
        np.asarray(input2, dtype=np.float32).reshape(16, DIM, QPU),
    ], axis=0)                                   # [32, 512, 1024]
    uh = units.astype(np.float16)
    ul = (units - uh.astype(np.float32)).astype(np.float16)

    mpT = np.ascontiguousarray(np.asarray(mempool, dtype=np.float32).T)  # [512, 2048]
    mh = mpT.astype(np.float16)
    ml = (mpT - mh.astype(np.float32)).astype(np.float16)
    mp16 = np.asarray(mempool, dtype=np.float32).astype(np.float16)
    ident = np.eye(128, dtype=np.float16)

    in_maps = []
    for k in range(NCORES):
        in_maps.append({
            "qh": np.ascontiguousarray(uh[4 * k:4 * (k + 1)]),
            "ql": np.ascontiguousarray(ul[4 * k:4 * (k + 1)]),
            "mh": mh, "ml": ml, "mp": mp16, "ident": ident,
        })
    return in_maps


def _assemble(results):
    # results: per-core dict with "out" [4096, 512]; unit 4k+j -> rows [1024j:1024(j+1)]
    outs = np.empty((32, DIM, QPU), dtype=np.float32)
    for k in range(NCORES):
        o = results[k]["out"]
        for j in range(UNITS_PER_CORE):
            outs[4 * k + j] = o[QPU * j:QPU * (j + 1), :].T
    out1 = outs[:16].reshape(16, DIM, 32, 32)
    out2 = outs[16:].reshape(16, DIM, 32, 32)
    return out1, out2


def kernel(input1, input2, mempool):
    nc = build_program()
    in_maps = _prep_inputs(input1, input2, mempool)
    res = run_bass_kernel_spmd(nc, in_maps, core_ids=list(range(NCORES)))
    return _assemble(res.results)


if __name__ == "__main__":
    rng = np.random.default_rng(0)
    i1 = rng.standard_normal((16, DIM, 32, 32)).astype(np.float32)
    i2 = rng.standard_normal((16, DIM, 32, 32)).astype(np.float32)
    mp = rng.uniform(-1 / np.sqrt(DIM), 1 / np.sqrt(DIM), (NITEM, DIM)).astype(np.float32)
    o1, o2 = kernel(i1, i2, mp)
    print("ok", o1.shape, o2.shape, o1.dtype)


# revision 5
# speedup vs baseline: 1.0126x; 1.0126x over previous
"""Trainium2 Bass kernel for nn_Memory (topk_masking).

Algorithm (per query row q of N=32768, item count 2048, K=10):
  logits l = q @ mempool.T
  e = exp(l); S = sum(e)                       (softmax, no max-shift: |l| <= ~3)
  top-10 selection on e via DVE max8 + match_replace + max8 -> t10 (10th largest)
  u = exp(e/S);  g = (e >= t10) * u;  Z = sum(g)
  out = (g @ mempool) / Z                      (masked dense matmul, no gather)

Precision: logits via 3-term fp16 split (qh@mh + qh@ml + ql@mh) -> ~fp32-grade
(sigma ~1e-7), so the selected top-10 set matches the fp32 reference exactly.
Second matmul in fp16 -> ~3e-4 relative output error.

Sharding: data-parallel over queries. 32 units of [512 dim x 1024 queries]
(16 batches x 2 inputs); each of 8 cores takes 4 units = 32 tiles of 128
queries. mempool (4MB) replicated per core. Host does layout marshalling only.
"""
import sys
sys.path.insert(0, '/opt/trn_rl_repo')

import numpy as np
import concourse.bacc as bacc
import concourse.mybir as mybir
import concourse.tile as tile
from concourse.bass_utils import run_bass_kernel_spmd

F32 = mybir.dt.float32
F16 = mybir.dt.float16

DIM = 512
NITEM = 2048
NCORES = 8
UNITS_PER_CORE = 4
QPU = 1024
TILES = UNITS_PER_CORE * QPU // 128
NEG = -1e30
EXP = mybir.ActivationFunctionType.Exp
COPY = mybir.ActivationFunctionType.Copy

_prog_cache = {}


def declare_io(nc, sfx="", internal=False):
    decl = (lambda n, s, d: nc.dram_tensor(n + sfx, s, d)) if internal else \
           (lambda n, s, d: nc.declare_dram_parameter(n + sfx, s, d, isOutput=False))
    d = {
        "qh": decl("qh", [UNITS_PER_CORE, DIM, QPU], F16),
        "ql": decl("ql", [UNITS_PER_CORE, DIM, QPU], F16),
        "mh": decl("mh", [DIM, NITEM], F16),
        "ml": decl("ml", [DIM, NITEM], F16),
        "mp": decl("mp", [NITEM, DIM], F16),
        "ident": decl("ident", [128, 128], F16),
    }
    if internal:
        d["out"] = nc.dram_tensor("out" + sfx, [UNITS_PER_CORE * QPU, DIM], F32)
    else:
        d["out"] = nc.declare_dram_parameter("out" + sfx, [UNITS_PER_CORE * QPU, DIM],
                                             F32, isOutput=True)
    return d


def emit(nc, tc, dram, reps=None):
    """Emit the full 32-tile workload (optionally wrapped in a For_i loop)."""
    with (
        tc.tile_pool(name="const", bufs=1) as cpool,
        tc.tile_pool(name="qin", bufs=3) as qpool,
        tc.tile_pool(name="work", bufs=2) as wpool,
        tc.tile_pool(name="outp", bufs=3) as opool,
        tc.tile_pool(name="ps_l", bufs=4, space="PSUM") as ps_l,
        tc.tile_pool(name="ps_t", bufs=2, space="PSUM") as ps_t,
        tc.tile_pool(name="ps_o", bufs=2, space="PSUM") as ps_o,
    ):
        mh_sb = cpool.tile([128, 4, NITEM], F16)
        ml_sb = cpool.tile([128, 4, NITEM], F16)
        mp_sb = cpool.tile([128, 16, DIM], F16)
        id_sb = cpool.tile([128, 128], F16)
        nc.sync.dma_start(mh_sb[:], dram["mh"][:].rearrange("(kc p) n -> p kc n", p=128))
        nc.sync.dma_start(ml_sb[:], dram["ml"][:].rearrange("(kc p) n -> p kc n", p=128))
        nc.sync.dma_start(mp_sb[:], dram["mp"][:].rearrange("(ic p) d -> p ic d", p=128))
        nc.sync.dma_start(id_sb[:], dram["ident"][:])

        def tile_body(t):
            u, tt = divmod(t, QPU // 128)
            qh_sb = qpool.tile([128, 4, 128], F16, tag="qh", name="qh_sb")
            ql_sb = qpool.tile([128, 4, 128], F16, tag="ql", name="ql_sb")
            nc.sync.dma_start(qh_sb[:], dram["qh"][u, :, 128 * tt:128 * (tt + 1)]
                              .rearrange("(kc p) f -> p kc f", p=128))
            nc.sync.dma_start(ql_sb[:], dram["ql"][u, :, 128 * tt:128 * (tt + 1)]
                              .rearrange("(kc p) f -> p kc f", p=128))

            e_sb = wpool.tile([128, NITEM], F32, tag="e", name="e_sb")
            S_p = wpool.tile([128, 4], F32, tag="Sp", name="S_p")
            cand8 = wpool.tile([128, 4, 8], F32, tag="cand8", name="cand8")
            # mm1 in four 512-item chunks; exp + chunk-max overlap later chunks
            for c in range(4):
                l_ps = ps_l.tile([128, 512], F32, tag="l", name="l_ps")
                col = 512 * c
                for kc in range(4):
                    nc.tensor.matmul(l_ps[:], qh_sb[:, kc, :],
                                     mh_sb[:, kc, col:col + 512],
                                     start=(kc == 0), stop=False)
                    nc.tensor.matmul(l_ps[:], qh_sb[:, kc, :],
                                     ml_sb[:, kc, col:col + 512],
                                     start=False, stop=False)
                for kc in range(4):
                    nc.tensor.matmul(l_ps[:], ql_sb[:, kc, :],
                                     mh_sb[:, kc, col:col + 512],
                                     start=False, stop=(kc == 3))
                nc.scalar.activation(e_sb[:, col:col + 512], l_ps[:],
                                     EXP, accum_out=S_p[:, c:c + 1])
                nc.vector.max(out=cand8[:, c, :], in_=e_sb[:, col:col + 512])

            S_sb = wpool.tile([128, 1], F32, tag="S", name="S_sb")
            nc.vector.tensor_reduce(S_sb[:], S_p[:], axis=mybir.AxisListType.X,
                                    op=mybir.AluOpType.add)

            # global top8 from per-chunk top8s; ranks 9-16 via per-chunk
            # match_replace (same top8 searched in each chunk) + chunk maxes
            top8 = wpool.tile([128, 8], F32, tag="top8", name="top8")
            next8 = wpool.tile([128, 8], F32, tag="next8", name="next8")
            nx8 = wpool.tile([128, 4, 8], F32, tag="nx8", name="nx8")
            em_sb = wpool.tile([128, NITEM], F32, tag="em", name="em_sb")
            nc.vector.max(out=top8[:], in_=cand8[:].rearrange("p a b -> p (a b)"))
            for c in range(4):
                col = 512 * c
                nc.vector.match_replace(out=em_sb[:, col:col + 512],
                                        in_to_replace=top8[:],
                                        in_values=e_sb[:, col:col + 512],
                                        imm_value=NEG)
                nc.vector.max(out=nx8[:, c, :], in_=em_sb[:, col:col + 512])
            nc.vector.max(out=next8[:], in_=nx8[:].rearrange("p a b -> p (a b)"))

            Sinv = wpool.tile([128, 1], F32, tag="Sinv", name="Sinv")
            nc.vector.reciprocal(Sinv[:], S_sb[:])
            u_sb = wpool.tile([128, NITEM], F16, tag="u", name="u_sb")
            nc.scalar.activation(u_sb[:], e_sb[:], EXP, scale=Sinv[:])

            g_sb = wpool.tile([128, NITEM], F16, tag="g", name="g_sb")
            Z_sb = wpool.tile([128, 1], F32, tag="Z", name="Z_sb")
            nc.vector.scalar_tensor_tensor(
                out=g_sb[:], in0=e_sb[:], scalar=next8[:, 1:2], in1=u_sb[:],
                op0=mybir.AluOpType.is_ge, op1=mybir.AluOpType.mult,
                accum_out=Z_sb[:])
            Zinv = wpool.tile([128, 1], F32, tag="Zinv", name="Zinv")
            nc.vector.reciprocal(Zinv[:], Z_sb[:])

            gt_sb = wpool.tile([128, 16, 128], F16, tag="gt", name="gt_sb")
            for hd in range(2):
                gt_ps = ps_t.tile([128, 1024], F16, tag="gt", name="gt_ps")
                for b in range(8):
                    blk = 8 * hd + b
                    nc.tensor.transpose(gt_ps[:, 128 * b:128 * (b + 1)],
                                        g_sb[:, 128 * blk:128 * (blk + 1)], id_sb[:])
                nc.scalar.copy(out=gt_sb[:, 8 * hd:8 * (hd + 1), :], in_=gt_ps[:])

            o_ps = ps_o.tile([128, DIM], F32, tag="o", name="o_ps")
            for ic in range(16):
                nc.tensor.matmul(o_ps[:], gt_sb[:, ic, :], mp_sb[:, ic, :],
                                 start=(ic == 0), stop=(ic == 15))
            o_sb = opool.tile([128, DIM], F32, tag="osb", name="o_sb")
            nc.scalar.activation(o_sb[:], o_ps[:], COPY, scale=Zinv[:])
            nc.sync.dma_start(dram["out"][128 * t:128 * (t + 1), :], o_sb[:])

        if reps is None:
            for t in range(TILES):
                tile_body(t)
        else:
            with tc.For_i(0, reps, 1):
                for t in range(TILES):
                    tile_body(t)


def build_program():
    if 'nc' in _prog_cache:
        return _prog_cache['nc']
    nc = bacc.Bacc()
    dram = declare_io(nc)
    with tile.TileContext(nc) as tc:
        emit(nc, tc, dram)
    nc.finalize()
    _prog_cache['nc'] = nc
    return nc


def _prep_inputs(input1, input2, mempool):
    units = np.concatenate([
        np.asarray(input1, dtype=np.float32).reshape(16, DIM, QPU),
        np.asarray(input2, dtype=np.float32).reshape(16, DIM, QPU),
    ], axis=0)
    uh = units.astype(np.float16)
    ul = (units - uh.astype(np.float32)).astype(np.float16)

    mpT = np.ascontiguousarray(np.asarray(mempool, dtype=np.float32).T)
    mh = mpT.astype(np.float16)
    ml = (mpT - mh.astype(np.float32)).astype(np.float16)
    mp16 = np.asarray(mempool, dtype=np.float32).astype(np.float16)
    ident = np.eye(128, dtype=np.float16)

    return [{
        "qh": np.ascontiguousarray(uh[4 * k:4 * (k + 1)]),
        "ql": np.ascontiguousarray(ul[4 * k:4 * (k + 1)]),
        "mh": mh, "ml": ml, "mp": mp16, "ident": ident,
    } for k in range(NCORES)]


def _assemble(results):
    outs = np.empty((32, DIM, QPU), dtype=np.float32)
    for k in range(NCORES):
        o = results[k]["out"]
        for j in range(UNITS_PER_CORE):
            outs[4 * k + j] = o[QPU * j:QPU * (j + 1), :].T
    return outs[:16].reshape(16, DIM, 32, 32), outs[16:].reshape(16, DIM, 32, 32)


def kernel(input1, input2, mempool):
    nc = build_program()
    in_maps = _prep_inputs(input1, input2, mempool)
    res = run_bass_kernel_spmd(nc, in_maps, core_ids=list(range(NCORES)))
    return _assemble(res.results)


if __name__ == "__main__":
    rng = np.random.default_rng(0)
    i1 = rng.standard_normal((16, DIM, 32, 32)).astype(np.float32)
    i2 = rng.standard_normal((16, DIM, 32, 32)).astype(np.float32)
    mp = rng.uniform(-1 / np.sqrt(DIM), 1 / np.sqrt(DIM), (NITEM, DIM)).astype(np.float32)
    o1, o2 = kernel(i1, i2, mp)
    print("ok", o1.shape, o2.shape, o1.dtype)
